# revision 71
# baseline (speedup 1.0000x reference)
"""Batch semi-hard triplet loss (cosine distance) on 8 Trainium2 NeuronCores.

Strategy (data-parallel over rows, per sharding hint):
  - Host: sort rows by label; normalize embeddings exactly in f32 (glue-scale
    O(B*D) work, 0.01% of the O(B^2*D) device FLOPs) and ship bf16; core c
    takes sorted rows [1024c, 1024(c+1)) in 8 exact 128-row M-tiles; columns
    rotated per core so its rows' class columns sit in the first banks.
  - Device (per core, uniform SPMD program):
      * per M-tile m: 16 bank matmuls. Class-column poison (-2) is applied
        ON THE PE via small rank-per-class accumulate matmuls (lhsT =
        -2*row-indicators, rhs = col-indicators) -- no mask adds anywhere.
      * t_p(m) (min positive-class dot) is precomputed one M-tile AHEAD
        from tiny narrow-window matmuls (+pois) and DVE window mins, so
        consumers fire the moment each bank's matmul lands; the f16
        -t_p row for the PE subtract is transposed [128,1]->[1,128] via a
        DRAM round-trip DMA.
      * threshold-max q = max{dot < t_p} via u = 1/(dot - t_p) (pole at
        the threshold flips the order; min u = dot closest below t_p):
          - Act banks 0-11: Reciprocal with per-partition -t_p bias,
            PSUM->bf16, in four 3-bank-wide streams (Act is the pacer);
          - DVE banks 12-15: PE pre-subtracts t_p (k=1 f16-row accumulate
            matmul), then a plain DVE Reciprocal (interleaved with the Act
            groups so both families pipeline independently);
          - r1 = min(u) in ONE fused DVE tensor_scalar with accum_out
            (4x mode), one M-tile behind.
        (GPSIMD can't read PSUM or do min on TRN2, so Pool sits out.)
      * per-family PSUM pools (Act 2x3 banks, DVE/mini 2x1) keep the two
        pipelines from ringing on PSUM reuse; a dummy recip preloads the
        Act table and dummy matmuls pre-warm the PE p-state during loads.
  - Host: q = t_p + 1/r1; per-row loss epilogue in f64; rows with no
    semi-hard candidate in the margin window (or near the branch boundary)
    are recomputed exactly in f32 numpy; mean over valid rows.
"""

import numpy as np
import ml_dtypes

B = 8192
D = 128
MARGIN = 0.2
NCORES = 8
NT = 512            # N-tile width (one PSUM bank of fp32)
N_NT = B // NT      # 16
MT = 128            # M-tile rows
NMT = B // NCORES // MT  # 8 m-tiles per core
GWA = 4             # banks per Act-family PSUM tile
POIS = -2.0         # class-column poison (exactly representable in bf16)

# bank families: Act does recip on banks 0..11; DVE on banks 12..15
# (GPSIMD cannot read PSUM on TRN2, so Pool is out of the main loop)
ACT_BANKS = list(range(0, 12))
DVE_BANKS = [12, 13, 14, 15]
NA = len(ACT_BANKS)
ND = len(DVE_BANKS)

BF16 = ml_dtypes.bfloat16

_CACHE = {}


# --------------------------------------------------------------------------
# host-side planning (pure layout, computed from labels)
# --------------------------------------------------------------------------
def _plan(labels: np.ndarray):
    order = np.argsort(labels, kind="stable")
    slab = labels[order]
    bounds = np.flatnonzero(np.r_[True, slab[1:] != slab[:-1], True])
    cls_start, cls_end = bounds[:-1], bounds[1:]
    row_s = np.empty(B, dtype=np.int64)
    row_e = np.empty(B, dtype=np.int64)
    for s, e in zip(cls_start, cls_end):
        row_s[s:e] = s
        row_e[s:e] = e

    rows_per_core = B // NCORES
    cores = []
    for c in range(NCORES):
        r0 = c * rows_per_core
        base = int(row_s[r0])  # start of first class -> no wraparound
        diag = []
        for m in range(NMT):
            rr = slice(r0 + m * MT, r0 + (m + 1) * MT)
            s = row_s[rr] - base
            e = row_e[rr] - base
            dts = sorted(set((s // NT).tolist()) | set(((e - 1) // NT).tolist()))
            diag.append(dts)
        cores.append(dict(r0=r0, base=base, diag=diag))
    # unify diag sets across cores so all 8 run one compiled program
    uni = [
        sorted(set().union(*[set(pc["diag"][m]) for pc in cores]))
        for m in range(NMT)
    ]
    for pc in cores:
        pc["diag"] = uni
    # per (m, diag tile): narrow column window [c0, c1) within the bank that
    # contains every class column of the tile's rows, across all cores
    wins = []
    for m in range(NMT):
        wm_ = []
        for d in uni[m]:
            c0, c1 = NT, 0
            for pc in cores:
                rr = slice(pc["r0"] + m * MT, pc["r0"] + (m + 1) * MT)
                s = np.maximum(row_s[rr] - pc["base"] - d * NT, 0)
                e = np.minimum(row_e[rr] - pc["base"] - d * NT, NT)
                ok = s < e
                if ok.any():
                    c0 = min(c0, int(s[ok].min()))
                    c1 = max(c1, int(e[ok].max()))
            if c1 <= c0:
                c0, c1 = 0, NT
            wm_.append((c0, c1))
        wins.append(wm_)
    # max classes per (m, diag-tile) block across cores (pois matmul k-dim)
    cp = 1
    for c in range(NCORES):
        pc = cores[c]
        r0, base = pc["r0"], pc["base"]
        for m in range(NMT):
            rr = slice(r0 + m * MT, r0 + (m + 1) * MT)
            ss = row_s[rr]
            for d in uni[m]:
                lo, hi = base + d * NT, base + (d + 1) * NT
                # classes whose column range intersects the bank
                cls = set()
                for g in range(rr.start, rr.stop):
                    if row_s[g] < hi and row_e[g] > lo:
                        cls.add(int(row_s[g]))
                cp = max(cp, len(cls))
    return dict(
        order=order, row_s=row_s, row_e=row_e, cores=cores, diag=uni,
        wins=wins, cp=cp,
    )


def _build_core_inputs(emb_norm: np.ndarray, plan, c: int):
    """emb_norm: label-sorted, unit-normalized embeddings (f32).
    Returns (xtn_rot [D,B], xbn [D,1024], pl [CP, nblk*MT],
    pr [CP, nblk*NT]) all bf16."""
    pc = plan["cores"][c]
    base, r0 = pc["base"], pc["r0"]
    rows_per_core = B // NCORES
    row_s, row_e = plan["row_s"], plan["row_e"]
    cp = plan["cp"]

    rot = np.r_[np.arange(base, B), np.arange(0, base)]
    xt_rot = np.ascontiguousarray(emb_norm[rot].T).astype(BF16)
    xb = np.ascontiguousarray(emb_norm[r0 : r0 + rows_per_core].T).astype(BF16)

    # poison matmul blocks: per (m, d in diag[m]):
    #   pl[k, i] = -2 if m-tile row i in class k else 0     [CP, MT]
    #   pr[k, j] = 1 if bank-d col j in class k else 0      [CP, NT]
    nblk = sum(len(d) for d in pc["diag"])
    pl = np.zeros((cp, nblk * MT), np.float32)
    pr = np.zeros((cp, nblk * NT), np.float32)
    bi = 0
    for m in range(NMT):
        for d in pc["diag"][m]:
            lo, hi = base + d * NT, base + (d + 1) * NT
            cls = {}
            for r in range(MT):
                g = r0 + m * MT + r
                s, e = int(row_s[g]), int(row_e[g])
                if s < hi and e > lo:
                    k = cls.setdefault(s, len(cls))
                    pl[k, bi * MT + r] = POIS
                    cs, ce = max(s - lo, 0), min(e - lo, NT)
                    pr[k, bi * NT + cs : bi * NT + ce] = 1.0
            assert len(cls) <= cp
            bi += 1
    pl = pl.astype(BF16)
    pr = pr.astype(BF16)

    return xt_rot, xb, pl, pr


# --------------------------------------------------------------------------
# device program
# --------------------------------------------------------------------------
def _raw_recip_bias(nc, out, in_, bias_ap):
    import concourse.mybir as mybir

    eng = nc.scalar
    ins = [
        eng.lower_ap(in_),
        eng.lower_ap(bias_ap),
        mybir.ImmediateValue(dtype=mybir.dt.float32, value=1.0),  # scale
        mybir.ImmediateValue(dtype=mybir.dt.float32, value=0.0),  # alpha
    ]
    return eng.add_instruction(
        mybir.InstActivation(
            name=f"I-{nc.next_id()}",
            func=mybir.ActivationFunctionType.Reciprocal,
            ins=ins,
            outs=[eng.lower_ap(out)],
        )
    )


def _build_bass(diag, wins, cp):
    import concourse.bacc as bacc
    import concourse.mybir as mybir
    from concourse.tile import TileContext

    f32 = mybir.dt.float32
    bf16 = mybir.dt.bfloat16
    f16 = mybir.dt.float16
    Alu = mybir.AluOpType
    NBC = NMT * MT  # xb columns (1024)
    nblk = sum(len(d) for d in diag)

    nc = bacc.Bacc("TRN2", target_bir_lowering=False, debug=False, num_devices=NCORES)

    xt_d = nc.dram_tensor("xt", [D, B], bf16, kind="ExternalInput").ap()
    xb_d = nc.dram_tensor("xb", [D, NBC], bf16, kind="ExternalInput").ap()
    pl_d = nc.dram_tensor("pl", [cp, nblk * MT], bf16, kind="ExternalInput").ap()
    pr_d = nc.dram_tensor("pr", [cp, nblk * NT], bf16, kind="ExternalInput").ap()
    out_d = nc.dram_tensor("out", [MT, 2 * NMT], f32, kind="ExternalOutput").ap()
    # per-m transposed -t_p rows (f16) round-trip through DRAM
    scr_d = nc.dram_tensor("scr", [NMT, MT], f16, kind="Internal").ap()

    # diag-block flat index per (m, d)
    blkof = {}
    bi = 0
    for m in range(NMT):
        for j, d in enumerate(diag[m]):
            blkof[(m, d)] = bi
            bi += 1

    with TileContext(nc) as tc:
        with (
            tc.tile_pool(name="big", bufs=1) as big,
            tc.tile_pool(name="upool", bufs=2) as upool,
            tc.tile_pool(name="sm", bufs=6) as smp,
            tc.tile_pool(name="psa", bufs=2, space="PSUM") as psa,
        ):
            # ---------------- setup: load pre-normalized inputs -------------
            # dummy recip FIRST so its act-table load isn't queued behind
            # the Act-queue DMA issues below
            dum = big.tile([1, 1], f32, tag="dum")
            nc.vector.memset(dum[:], 1.0)
            dumo = big.tile([1, 1], bf16, tag="dumo")
            _raw_recip_bias(nc, dumo[:], dum[:], dum[:])
            # xbn + first xtn chunks on the SP HWDGE queue; the back half of
            # xtn on the Activation HWDGE queue (two queues load in parallel)
            xbn = big.tile([D, NBC], bf16, tag="xbn")
            nc.sync.dma_start(xbn[:], xb_d)
            pl = big.tile([cp, nblk * MT], bf16, tag="pl")
            nc.sync.dma_start(pl[:], pl_d)
            pr = big.tile([cp, nblk * NT], bf16, tag="pr")
            nc.sync.dma_start(pr[:], pr_d)
            xtn = big.tile([D, B], bf16, tag="xtn")
            for j in range(4):
                sl = slice(j * (B // 8), (j + 1) * (B // 8))
                nc.sync.dma_start(xtn[:, sl], xt_d[:, sl])
            for j in range(4, 8):
                sl = slice(j * (B // 8), (j + 1) * (B // 8))
                nc.scalar.dma_start(xtn[:, sl], xt_d[:, sl])

            outb = big.tile([MT, 2 * NMT], f32, tag="outb")
            ntpall = big.tile([MT, NMT], f32, tag="ntpall")
            ones = big.tile([1, NT], f16, tag="ones")
            nc.vector.memset(ones[:], 1.0)
            onesr = big.tile([1, MT], f16, tag="onesr")
            nc.vector.memset(onesr[:], 1.0)
            # pre-warm the PE p-state with a ~3us dummy matmul streak
            for _ in range(2):
                warm = psa.tile([MT, GWA * NT], f32, tag="w")
                for _k in range(7):
                    nc.tensor.matmul(warm[:, 0:NT], onesr[:], ones[:])

            # ---------------- main loop over M-tiles ----------------
            # Act-family groups of GWA banks; DVE-family single banks
            agroups = [
                ACT_BANKS[i : i + GWA] for i in range(0, NA, GWA)
            ]

            def emit_tp(m):
                """Narrow-window diag matmuls (+pois) into small PSUM tiles,
                then the t_p chain on DVE."""
                dts = diag[m]
                lhsT = xbn[:, m * MT : (m + 1) * MT]
                tpp = outb[:, m : m + 1]
                ntp = ntpall[:, m : m + 1]
                mini = psa.tile([MT, GWA * NT], f32, tag="w")
                for j, d in enumerate(dts):
                    c0, c1 = wins[m][j]
                    w = c1 - c0
                    i = blkof[(m, d)]
                    nc.tensor.matmul(
                        mini[:, j * NT : j * NT + w],
                        lhsT, xtn[:, d * NT + c0 : d * NT + c1],
                        start=True, stop=False,
                    )
                    nc.tensor.matmul(
                        mini[:, j * NT : j * NT + w],
                        pl[:, i * MT : (i + 1) * MT],
                        pr[:, i * NT + c0 : i * NT + c1],
                        start=False, stop=True,
                    )
                ndts = len(dts)
                posm = smp.tile([MT, max(ndts, 1)], f32, tag="posm")
                for j, d in enumerate(dts):
                    c0, c1 = wins[m][j]
                    nc.vector.tensor_reduce(
                        posm[:, j : j + 1],
                        mini[:, j * NT : j * NT + (c1 - c0)],
                        axis=mybir.AxisListType.X, op=Alu.min,
                    )
                if ndts == 1:
                    minpos = posm[:, 0:1]
                else:
                    mp = smp.tile([MT, 1], f32, tag="minpos")
                    nc.vector.tensor_reduce(
                        mp[:], posm[:], axis=mybir.AxisListType.X, op=Alu.min
                    )
                    minpos = mp[:]
                # t_p = min(minpos - POIS, 1)
                nc.vector.tensor_scalar(
                    tpp, minpos, -POIS, 1.0, Alu.add, Alu.min
                )
                nc.vector.tensor_scalar_mul(ntp, tpp, -1.0)
                # transposed -t_p row (f16) for the PE subtract on DVE banks:
                # col -> row via a DRAM round-trip (2 tiny DMAs, 1 m ahead)
                ntph = smp.tile([MT, 1], f16, tag="ntph", bufs=3)
                with nc.allow_low_precision(reason="f16 threshold row"):
                    nc.vector.tensor_scalar_mul(ntph[:], tpp, -1.0)
                nc.sync.dma_start(scr_d[m : m + 1, :], ntph[:])
                tprow = smp.tile([1, MT], f16, tag="tprow", bufs=3)
                nc.sync.dma_start(tprow[:], scr_d[m : m + 1, :])
                tprows[m] = tprow

            def mm_bank(wg, ofs, t, m, lhsT, stop=True):
                """main matmul for N-tile t into wg[:, ofs*NT:...], plus the
                class-poison accumulate matmul on diag banks."""
                if t in diag[m]:
                    nc.tensor.matmul(
                        wg[:, ofs * NT : (ofs + 1) * NT],
                        lhsT, xtn[:, t * NT : (t + 1) * NT],
                        start=True, stop=False,
                    )
                    i = blkof[(m, t)]
                    nc.tensor.matmul(
                        wg[:, ofs * NT : (ofs + 1) * NT],
                        pl[:, i * MT : (i + 1) * MT],
                        pr[:, i * NT : (i + 1) * NT],
                        start=False, stop=stop,
                    )
                else:
                    nc.tensor.matmul(
                        wg[:, ofs * NT : (ofs + 1) * NT],
                        lhsT, xtn[:, t * NT : (t + 1) * NT],
                        start=True, stop=stop,
                    )

            tprows = {}
            emit_tp(0)

            pending_min = None  # (u, m) of previous M-tile

            def emit_mins(u, m):
                # fused elementwise+accumulate-min: r1 = min(u) on DVE (4x)
                uj = upool.tile([MT, N_NT * NT], bf16, tag="uj")
                with nc.allow_low_precision(reason="u is bf16 by design"):
                    nc.vector.tensor_scalar(
                        uj[:], u[:], 1.0, None, Alu.mult, Alu.min,
                        accum_out=outb[:, NMT + m : NMT + m + 1],
                    )

            for m in range(NMT):
                lhsT = xbn[:, m * MT : (m + 1) * MT]
                ntp = ntpall[:, m : m + 1]     # -t_p ptr (f32)
                u = upool.tile([MT, N_NT * NT], bf16, tag="u")

                def emit_a(banks):
                    wg = psa.tile([MT, GWA * NT], f32, tag="w")
                    for k, t in enumerate(banks):
                        mm_bank(wg, k, t, m, lhsT)
                    b0 = banks[0]
                    _raw_recip_bias(
                        nc,
                        u[:, b0 * NT : (b0 + len(banks)) * NT],
                        wg[:, 0 : len(banks) * NT],
                        ntp,
                    )

                def emit_d():
                    # all 4 DVE banks in one 4-bank tile: PE pre-subtracts
                    # t_p (k=1 f16 rows), then ONE wide DVE reciprocal
                    wg = psa.tile([MT, GWA * NT], f32, tag="w")
                    for k, t in enumerate(DVE_BANKS):
                        mm_bank(wg, k, t, m, lhsT, stop=False)
                        nc.tensor.matmul(
                            wg[:, k * NT : (k + 1) * NT], tprows[m][:],
                            ones[:], start=False, stop=True,
                        )
                    with nc.allow_low_precision(reason="u is bf16 by design"):
                        nc.vector.reciprocal(
                            u[:, DVE_BANKS[0] * NT :], wg[:]
                        )

                # Act groups and the DVE tile pipeline independently; the
                # next M-tile's t_p chain is issued early so its minis
                # aren't stuck behind all of this tile's fills on PE
                emit_a(agroups[0])
                emit_d()
                if m + 1 < NMT:
                    emit_tp(m + 1)
                emit_a(agroups[1])
                emit_a(agroups[2])

                if pending_min is not None:
                    emit_mins(*pending_min)
                pending_min = (u, m)

            # final M-tile: split the min so the first half overlaps the
            # tail recips, shrinking the serial epilogue
            u, m = pending_min
            H = 9 * NT  # banks 0..8 done after the first three Act groups
            uj = upool.tile([MT, N_NT * NT], bf16, tag="uj")
            ra = smp.tile([MT, 2], f32, tag="ra")
            with nc.allow_low_precision(reason="u is bf16 by design"):
                nc.vector.tensor_scalar(
                    uj[:, 0:H], u[:, 0:H], 1.0, None, Alu.mult, Alu.min,
                    accum_out=ra[:, 0:1],
                )
                nc.vector.tensor_scalar(
                    uj[:, H:], u[:, H:], 1.0, None, Alu.mult, Alu.min,
                    accum_out=ra[:, 1:2],
                )
            nc.vector.tensor_tensor(
                outb[:, NMT + m : NMT + m + 1], ra[:, 0:1], ra[:, 1:2],
                Alu.min,
            )

            nc.sync.dma_start(out_d, outb[:])

    nc.compile()
    return nc


# --------------------------------------------------------------------------
# entry point
# --------------------------------------------------------------------------
def _prepare(embeddings, labels):
    emb = np.asarray(embeddings, dtype=np.float32)
    lab = np.asarray(labels).astype(np.int64)
    plan = _plan(lab)
    emb_sorted = emb[plan["order"]]
    norm = np.linalg.norm(emb_sorted, axis=1, keepdims=True)
    emb_norm = emb_sorted / np.maximum(norm, 1e-12)
    cores = [_build_core_inputs(emb_norm, plan, c) for c in range(NCORES)]
    return emb, lab, plan, cores


def _host_reduce(emb, lab, plan, outs):
    """outs: per core {"out": [128, 16] f32} (cols 0-7 t_p, 8-15 r1)."""
    order = plan["order"]
    slab = lab[order]
    rows_per_core = B // NCORES

    t_p = np.zeros(B, np.float64)
    r1 = np.zeros(B, np.float64)
    for c in range(NCORES):
        o = np.asarray(outs[c]["out"], np.float64)
        for m in range(NMT):
            rr = slice(c * rows_per_core + m * MT, c * rows_per_core + (m + 1) * MT)
            t_p[rr] = o[:, m]
            r1[rr] = o[:, NMT + m]

    with np.errstate(divide="ignore", invalid="ignore"):
        q = t_p + 1.0 / r1
    d_ap = 1.0 - t_p
    d_semi = 1.0 - q
    lo = t_p - MARGIN

    # validity from class counts
    _, inv, counts = np.unique(slab, return_inverse=True, return_counts=True)
    cnt_row = counts[inv]
    valid = (cnt_row >= 2) & (cnt_row <= B - 1)

    EDGE = 1e-3
    semi_ok = (q > lo + EDGE) & (q < t_p) & np.isfinite(q) & (r1 < 0)
    redo = valid & ~semi_ok

    per_row = np.where(valid, np.maximum(d_ap - d_semi + MARGIN, 0.0), 0.0)

    if redo.any():
        e = emb / np.maximum(
            np.linalg.norm(emb, axis=1, keepdims=True), 1e-12
        )
        idx = order[np.flatnonzero(redo)]  # original row indices
        for g, i in zip(np.flatnonzero(redo), idx):
            dot = (e[i] @ e.T).astype(np.float32)
            dist = np.clip(1.0 - dot, 0.0, None)
            pos = (lab == lab[i])
            pos[i] = False
            neg = lab != lab[i]
            dap = dist[pos].max()
            semi = neg & (dist > dap) & (dist < dap + MARGIN)
            if semi.any():
                dan = dist[semi].min()
            else:
                dan = dist[neg].min()
            per_row[g] = max(dap - dan + MARGIN, 0.0)

    num_valid = max(int(valid.sum()), 1)
    loss = per_row[valid].sum() / num_valid
    return np.array(loss, dtype=np.float32)


def kernel_run(embeddings, labels, trace=False):
    import concourse.bass_utils as bass_utils

    emb, lab, plan, cores = _prepare(embeddings, labels)
    diag = plan["diag"]
    wins = plan["wins"]
    cp = plan["cp"]
    key = (
        tuple(tuple(d) for d in diag),
        tuple(tuple(w) for w in wins),
        cp,
    )
    if key not in _CACHE:
        _CACHE[key] = _build_bass(diag, wins, cp)
    nc = _CACHE[key]
    in_maps = [
        {"xt": np.ascontiguousarray(c[0]), "xb": np.ascontiguousarray(c[1]),
         "pl": np.ascontiguousarray(c[2]), "pr": np.ascontiguousarray(c[3])}
        for c in cores
    ]
    res = bass_utils.run_bass_kernel_spmd(
        nc, in_maps, core_ids=list(range(NCORES)), trace=trace
    )
    loss = _host_reduce(emb, lab, plan, res.results)
    return loss, res


def kernel(embeddings, labels):
    loss, _ = kernel_run(embeddings, labels)
    return loss


# revision 72
# speedup vs baseline: 1.4226x; 1.4226x over previous
"""Batch semi-hard triplet loss (cosine distance) on 8 Trainium2 NeuronCores.

Strategy (data-parallel over rows, per sharding hint):
  - Host: sort rows by label; normalize embeddings exactly in f32 (glue-scale
    O(B*D) work, 0.01% of the O(B^2*D) device FLOPs) and ship bf16; core c
    takes sorted rows [1024c, 1024(c+1)) in 8 exact 128-row M-tiles; columns
    rotated per core so its rows' class columns sit in the first banks.
  - Device (per core, uniform SPMD program):
      * per M-tile m: 16 bank matmuls. Class-column poison (-2) is applied
        ON THE PE via small rank-per-class accumulate matmuls (lhsT =
        -2*row-indicators, rhs = col-indicators) -- no mask adds anywhere.
      * t_p(m) (min positive-class dot) is precomputed one M-tile AHEAD
        from tiny narrow-window matmuls (+pois) and DVE window mins, so
        consumers fire the moment each bank's matmul lands; the f16
        -t_p row for the PE subtract is transposed [128,1]->[1,128] via a
        DRAM round-trip DMA.
      * threshold-max q = max{dot < t_p} via u = 1/(dot - t_p) (pole at
        the threshold flips the order; min u = dot closest below t_p):
          - Act banks 0-11: Reciprocal with per-partition -t_p bias,
            PSUM->bf16, in four 3-bank-wide streams (Act is the pacer);
          - DVE banks 12-15: PE pre-subtracts t_p (k=1 f16-row accumulate
            matmul), then a plain DVE Reciprocal (interleaved with the Act
            groups so both families pipeline independently);
          - r1 = min(u) in ONE fused DVE tensor_scalar with accum_out
            (4x mode), one M-tile behind.
        (GPSIMD can't read PSUM or do min on TRN2, so Pool sits out.)
      * per-family PSUM pools (Act 2x3 banks, DVE/mini 2x1) keep the two
        pipelines from ringing on PSUM reuse; a dummy recip preloads the
        Act table and dummy matmuls pre-warm the PE p-state during loads.
  - Host: q = t_p + 1/r1; per-row loss epilogue in f64; rows with no
    semi-hard candidate in the margin window (or near the branch boundary)
    are recomputed exactly in f32 numpy; mean over valid rows.
"""

import numpy as np
import ml_dtypes

B = 8192
D = 128
MARGIN = 0.2
NCORES = 8
NT = 512            # N-tile width (one PSUM bank of fp32)
N_NT = B // NT      # 16
MT = 128            # M-tile rows
NMT = B // NCORES // MT  # 8 m-tiles per core
GWA = 3             # banks per Act-family PSUM tile
POIS = -2.0         # class-column poison (exactly representable in bf16)

# bank families: Act does recip on banks 0..11; DVE on banks 12..15
# (GPSIMD cannot read PSUM on TRN2, so Pool is out of the main loop)
ACT_BANKS = list(range(0, 12))
DVE_BANKS = [12, 13, 14, 15]
NA = len(ACT_BANKS)
ND = len(DVE_BANKS)

BF16 = ml_dtypes.bfloat16

_CACHE = {}


# --------------------------------------------------------------------------
# host-side planning (pure layout, computed from labels)
# --------------------------------------------------------------------------
def _plan(labels: np.ndarray):
    order = np.argsort(labels, kind="stable")
    slab = labels[order]
    bounds = np.flatnonzero(np.r_[True, slab[1:] != slab[:-1], True])
    cls_start, cls_end = bounds[:-1], bounds[1:]
    row_s = np.empty(B, dtype=np.int64)
    row_e = np.empty(B, dtype=np.int64)
    for s, e in zip(cls_start, cls_end):
        row_s[s:e] = s
        row_e[s:e] = e

    rows_per_core = B // NCORES
    cores = []
    for c in range(NCORES):
        r0 = c * rows_per_core
        base = int(row_s[r0])  # start of first class -> no wraparound
        diag = []
        for m in range(NMT):
            rr = slice(r0 + m * MT, r0 + (m + 1) * MT)
            s = row_s[rr] - base
            e = row_e[rr] - base
            dts = sorted(set((s // NT).tolist()) | set(((e - 1) // NT).tolist()))
            diag.append(dts)
        cores.append(dict(r0=r0, base=base, diag=diag))
    # unify diag sets across cores so all 8 run one compiled program
    uni = [
        sorted(set().union(*[set(pc["diag"][m]) for pc in cores]))
        for m in range(NMT)
    ]
    for pc in cores:
        pc["diag"] = uni
    # per (m, diag tile): narrow column window [c0, c1) within the bank that
    # contains every class column of the tile's rows, across all cores
    wins = []
    for m in range(NMT):
        wm_ = []
        for d in uni[m]:
            c0, c1 = NT, 0
            for pc in cores:
                rr = slice(pc["r0"] + m * MT, pc["r0"] + (m + 1) * MT)
                s = np.maximum(row_s[rr] - pc["base"] - d * NT, 0)
                e = np.minimum(row_e[rr] - pc["base"] - d * NT, NT)
                ok = s < e
                if ok.any():
                    c0 = min(c0, int(s[ok].min()))
                    c1 = max(c1, int(e[ok].max()))
            if c1 <= c0:
                c0, c1 = 0, NT
            wm_.append((c0, c1))
        wins.append(wm_)
    # max classes per (m, diag-tile) block across cores (pois matmul k-dim)
    cp = 1
    for c in range(NCORES):
        pc = cores[c]
        r0, base = pc["r0"], pc["base"]
        for m in range(NMT):
            rr = slice(r0 + m * MT, r0 + (m + 1) * MT)
            ss = row_s[rr]
            for d in uni[m]:
                lo, hi = base + d * NT, base + (d + 1) * NT
                # classes whose column range intersects the bank
                cls = set()
                for g in range(rr.start, rr.stop):
                    if row_s[g] < hi and row_e[g] > lo:
                        cls.add(int(row_s[g]))
                cp = max(cp, len(cls))
    return dict(
        order=order, row_s=row_s, row_e=row_e, cores=cores, diag=uni,
        wins=wins, cp=cp,
    )


def _build_core_inputs(emb_norm: np.ndarray, plan, c: int):
    """emb_norm: label-sorted, unit-normalized embeddings (f32).
    Returns (xtn_rot [D,B], xbn [D,1024], pl [CP, nblk*MT],
    pr [CP, nblk*NT]) all bf16."""
    pc = plan["cores"][c]
    base, r0 = pc["base"], pc["r0"]
    rows_per_core = B // NCORES
    row_s, row_e = plan["row_s"], plan["row_e"]
    cp = plan["cp"]

    rot = np.r_[np.arange(base, B), np.arange(0, base)]
    xt_rot = np.ascontiguousarray(emb_norm[rot].T).astype(BF16)
    xb = np.ascontiguousarray(emb_norm[r0 : r0 + rows_per_core].T).astype(BF16)

    # poison matmul blocks: per (m, d in diag[m]):
    #   pl[k, i] = -2 if m-tile row i in class k else 0     [CP, MT]
    #   pr[k, j] = 1 if bank-d col j in class k else 0      [CP, NT]
    nblk = sum(len(d) for d in pc["diag"])
    pl = np.zeros((cp, nblk * MT), np.float32)
    pr = np.zeros((cp, nblk * NT), np.float32)
    bi = 0
    for m in range(NMT):
        for d in pc["diag"][m]:
            lo, hi = base + d * NT, base + (d + 1) * NT
            cls = {}
            for r in range(MT):
                g = r0 + m * MT + r
                s, e = int(row_s[g]), int(row_e[g])
                if s < hi and e > lo:
                    k = cls.setdefault(s, len(cls))
                    pl[k, bi * MT + r] = POIS
                    cs, ce = max(s - lo, 0), min(e - lo, NT)
                    pr[k, bi * NT + cs : bi * NT + ce] = 1.0
            assert len(cls) <= cp
            bi += 1
    pl = pl.astype(BF16)
    pr = pr.astype(BF16)

    return xt_rot, xb, pl, pr


# --------------------------------------------------------------------------
# device program
# --------------------------------------------------------------------------
def _raw_recip_bias(nc, out, in_, bias_ap):
    import concourse.mybir as mybir

    eng = nc.scalar
    ins = [
        eng.lower_ap(in_),
        eng.lower_ap(bias_ap),
        mybir.ImmediateValue(dtype=mybir.dt.float32, value=1.0),  # scale
        mybir.ImmediateValue(dtype=mybir.dt.float32, value=0.0),  # alpha
    ]
    return eng.add_instruction(
        mybir.InstActivation(
            name=f"I-{nc.next_id()}",
            func=mybir.ActivationFunctionType.Reciprocal,
            ins=ins,
            outs=[eng.lower_ap(out)],
        )
    )


def _build_bass(diag, wins, cp):
    import concourse.bacc as bacc
    import concourse.mybir as mybir
    from concourse.tile import TileContext

    f32 = mybir.dt.float32
    bf16 = mybir.dt.bfloat16
    f16 = mybir.dt.float16
    Alu = mybir.AluOpType
    NBC = NMT * MT  # xb columns (1024)
    nblk = sum(len(d) for d in diag)

    nc = bacc.Bacc("TRN2", target_bir_lowering=False, debug=False, num_devices=NCORES)

    xt_d = nc.dram_tensor("xt", [D, B], bf16, kind="ExternalInput").ap()
    xb_d = nc.dram_tensor("xb", [D, NBC], bf16, kind="ExternalInput").ap()
    pl_d = nc.dram_tensor("pl", [cp, nblk * MT], bf16, kind="ExternalInput").ap()
    pr_d = nc.dram_tensor("pr", [cp, nblk * NT], bf16, kind="ExternalInput").ap()
    out_d = nc.dram_tensor("out", [MT, 2 * NMT], f32, kind="ExternalOutput").ap()
    # per-m transposed -t_p rows (f16) round-trip through DRAM
    scr_d = nc.dram_tensor("scr", [NMT, MT], f16, kind="Internal").ap()

    # diag-block flat index per (m, d)
    blkof = {}
    bi = 0
    for m in range(NMT):
        for j, d in enumerate(diag[m]):
            blkof[(m, d)] = bi
            bi += 1

    with TileContext(nc) as tc:
        with (
            tc.tile_pool(name="big", bufs=1) as big,
            tc.tile_pool(name="upool", bufs=2) as upool,
            tc.tile_pool(name="sm", bufs=6) as smp,
            tc.tile_pool(name="psa", bufs=2, space="PSUM") as psa,
            tc.tile_pool(name="psp", bufs=2, space="PSUM") as psw,
        ):
            # ---------------- setup: load pre-normalized inputs -------------
            # dummy recip FIRST so its act-table load isn't queued behind
            # the Act-queue DMA issues below
            dum = big.tile([1, 1], f32, tag="dum")
            nc.vector.memset(dum[:], 1.0)
            dumo = big.tile([1, 1], bf16, tag="dumo")
            _raw_recip_bias(nc, dumo[:], dum[:], dum[:])
            # xbn + first xtn chunks on the SP HWDGE queue; the back half of
            # xtn on the Activation HWDGE queue (two queues load in parallel)
            xbn = big.tile([D, NBC], bf16, tag="xbn")
            nc.sync.dma_start(xbn[:], xb_d)
            pl = big.tile([cp, nblk * MT], bf16, tag="pl")
            nc.sync.dma_start(pl[:], pl_d)
            pr = big.tile([cp, nblk * NT], bf16, tag="pr")
            nc.sync.dma_start(pr[:], pr_d)
            xtn = big.tile([D, B], bf16, tag="xtn")
            for j in range(4):
                sl = slice(j * (B // 8), (j + 1) * (B // 8))
                nc.sync.dma_start(xtn[:, sl], xt_d[:, sl])
            for j in range(4, 8):
                sl = slice(j * (B // 8), (j + 1) * (B // 8))
                nc.scalar.dma_start(xtn[:, sl], xt_d[:, sl])

            outb = big.tile([MT, 2 * NMT], f32, tag="outb")
            ntpall = big.tile([MT, NMT], f32, tag="ntpall")
            ones = big.tile([1, NT], f16, tag="ones")
            nc.vector.memset(ones[:], 1.0)
            onesr = big.tile([1, MT], f16, tag="onesr")
            nc.vector.memset(onesr[:], 1.0)
            # pre-warm the PE p-state with a ~3us dummy matmul streak
            for _ in range(2):
                warm = psw.tile([MT, NT], f32, tag="w")
                for _k in range(7):
                    nc.tensor.matmul(warm[:], onesr[:], ones[:])

            # ---------------- main loop over M-tiles ----------------
            # Act-family groups of GWA banks; DVE-family single banks
            agroups = [
                ACT_BANKS[i : i + GWA] for i in range(0, NA, GWA)
            ]

            def emit_tp(m):
                """Narrow-window diag matmuls (+pois) into small PSUM tiles,
                then the t_p chain on DVE."""
                dts = diag[m]
                lhsT = xbn[:, m * MT : (m + 1) * MT]
                tpp = outb[:, m : m + 1]
                ntp = ntpall[:, m : m + 1]
                minis = []
                for j, d in enumerate(dts):
                    mini = psw.tile([MT, NT], f32, tag="w")
                    minis.append(mini)
                    c0, c1 = wins[m][j]
                    w = c1 - c0
                    i = blkof[(m, d)]
                    nc.tensor.matmul(
                        mini[:, 0:w],
                        lhsT, xtn[:, d * NT + c0 : d * NT + c1],
                        start=True, stop=False,
                    )
                    nc.tensor.matmul(
                        mini[:, 0:w],
                        pl[:, i * MT : (i + 1) * MT],
                        pr[:, i * NT + c0 : i * NT + c1],
                        start=False, stop=True,
                    )
                ndts = len(dts)
                posm = smp.tile([MT, max(ndts, 1)], f32, tag="posm")
                for j, d in enumerate(dts):
                    c0, c1 = wins[m][j]
                    nc.vector.tensor_reduce(
                        posm[:, j : j + 1],
                        minis[j][:, 0 : c1 - c0],
                        axis=mybir.AxisListType.X, op=Alu.min,
                    )
                if ndts == 1:
                    minpos = posm[:, 0:1]
                else:
                    mp = smp.tile([MT, 1], f32, tag="minpos")
                    nc.vector.tensor_reduce(
                        mp[:], posm[:], axis=mybir.AxisListType.X, op=Alu.min
                    )
                    minpos = mp[:]
                # t_p = min(minpos - POIS, 1)
                nc.vector.tensor_scalar(
                    tpp, minpos, -POIS, 1.0, Alu.add, Alu.min
                )
                nc.vector.tensor_scalar_mul(ntp, tpp, -1.0)
                # transposed -t_p row (f16) for the PE subtract on DVE banks:
                # col -> row via a DRAM round-trip (2 tiny DMAs, 1 m ahead)
                ntph = smp.tile([MT, 1], f16, tag="ntph", bufs=3)
                with nc.allow_low_precision(reason="f16 threshold row"):
                    nc.vector.tensor_scalar_mul(ntph[:], tpp, -1.0)
                nc.sync.dma_start(scr_d[m : m + 1, :], ntph[:])
                tprow = smp.tile([1, MT], f16, tag="tprow", bufs=3)
                nc.sync.dma_start(tprow[:], scr_d[m : m + 1, :])
                tprows[m] = tprow

            def mm_bank(wg, ofs, t, m, lhsT, stop=True):
                """main matmul for N-tile t into wg[:, ofs*NT:...], plus the
                class-poison accumulate matmul on diag banks."""
                if t in diag[m]:
                    nc.tensor.matmul(
                        wg[:, ofs * NT : (ofs + 1) * NT],
                        lhsT, xtn[:, t * NT : (t + 1) * NT],
                        start=True, stop=False,
                    )
                    i = blkof[(m, t)]
                    nc.tensor.matmul(
                        wg[:, ofs * NT : (ofs + 1) * NT],
                        pl[:, i * MT : (i + 1) * MT],
                        pr[:, i * NT : (i + 1) * NT],
                        start=False, stop=stop,
                    )
                else:
                    nc.tensor.matmul(
                        wg[:, ofs * NT : (ofs + 1) * NT],
                        lhsT, xtn[:, t * NT : (t + 1) * NT],
                        start=True, stop=stop,
                    )

            tprows = {}
            emit_tp(0)

            pending_min = None  # (u, m) of previous M-tile

            def emit_mins(u, m):
                # fused elementwise+accumulate-min: r1 = min(u) on DVE (4x)
                uj = upool.tile([MT, N_NT * NT], bf16, tag="uj")
                with nc.allow_low_precision(reason="u is bf16 by design"):
                    nc.vector.tensor_scalar(
                        uj[:], u[:], 1.0, None, Alu.mult, Alu.min,
                        accum_out=outb[:, NMT + m : NMT + m + 1],
                    )

            for m in range(NMT):
                lhsT = xbn[:, m * MT : (m + 1) * MT]
                ntp = ntpall[:, m : m + 1]     # -t_p ptr (f32)
                u = upool.tile([MT, N_NT * NT], bf16, tag="u")

                def emit_a(banks):
                    wg = psa.tile([MT, GWA * NT], f32, tag="w")
                    for k, t in enumerate(banks):
                        mm_bank(wg, k, t, m, lhsT)
                    b0 = banks[0]
                    _raw_recip_bias(
                        nc,
                        u[:, b0 * NT : (b0 + len(banks)) * NT],
                        wg[:, 0 : len(banks) * NT],
                        ntp,
                    )

                def emit_d(t):
                    # PE pre-subtracts t_p (k=1 f16 row), then DVE reciprocal
                    wg = psw.tile([MT, NT], f32, tag="w")
                    mm_bank(wg, 0, t, m, lhsT, stop=False)
                    nc.tensor.matmul(
                        wg[:], tprows[m][:], ones[:],
                        start=False, stop=True,
                    )
                    with nc.allow_low_precision(reason="u is bf16 by design"):
                        nc.vector.reciprocal(
                            u[:, t * NT : (t + 1) * NT], wg[:]
                        )

                # interleave the two families so Act and DVE pipeline
                # independently and the PE weaves between them; the next
                # M-tile's t_p chain is issued early so its minis aren't
                # stuck behind all of this tile's fills on the PE queue
                emit_a(agroups[0])
                emit_d(DVE_BANKS[0])
                if m + 1 < NMT:
                    emit_tp(m + 1)
                emit_a(agroups[1])
                emit_d(DVE_BANKS[1])
                emit_a(agroups[2])
                emit_d(DVE_BANKS[2])
                emit_a(agroups[3])
                emit_d(DVE_BANKS[3])

                if pending_min is not None:
                    emit_mins(*pending_min)
                pending_min = (u, m)

            # final M-tile: split the min so the first half overlaps the
            # tail recips, shrinking the serial epilogue
            u, m = pending_min
            H = 9 * NT  # banks 0..8 done after the first three Act groups
            uj = upool.tile([MT, N_NT * NT], bf16, tag="uj")
            ra = smp.tile([MT, 2], f32, tag="ra")
            with nc.allow_low_precision(reason="u is bf16 by design"):
                nc.vector.tensor_scalar(
                    uj[:, 0:H], u[:, 0:H], 1.0, None, Alu.mult, Alu.min,
                    accum_out=ra[:, 0:1],
                )
                nc.vector.tensor_scalar(
                    uj[:, H:], u[:, H:], 1.0, None, Alu.mult, Alu.min,
                    accum_out=ra[:, 1:2],
                )
            nc.vector.tensor_tensor(
                outb[:, NMT + m : NMT + m + 1], ra[:, 0:1], ra[:, 1:2],
                Alu.min,
            )

            nc.sync.dma_start(out_d, outb[:])

    nc.compile()
    return nc


# --------------------------------------------------------------------------
# entry point
# --------------------------------------------------------------------------
def _prepare(embeddings, labels):
    emb = np.asarray(embeddings, dtype=np.float32)
    lab = np.asarray(labels).astype(np.int64)
    plan = _plan(lab)
    emb_sorted = emb[plan["order"]]
    norm = np.linalg.norm(emb_sorted, axis=1, keepdims=True)
    emb_norm = emb_sorted / np.maximum(norm, 1e-12)
    cores = [_build_core_inputs(emb_norm, plan, c) for c in range(NCORES)]
    return emb, lab, plan, cores


def _host_reduce(emb, lab, plan, outs):
    """outs: per core {"out": [128, 16] f32} (cols 0-7 t_p, 8-15 r1)."""
    order = plan["order"]
    slab = lab[order]
    rows_per_core = B // NCORES

    t_p = np.zeros(B, np.float64)
    r1 = np.zeros(B, np.float64)
    for c in range(NCORES):
        o = np.asarray(outs[c]["out"], np.float64)
        for m in range(NMT):
            rr = slice(c * rows_per_core + m * MT, c * rows_per_core + (m + 1) * MT)
            t_p[rr] = o[:, m]
            r1[rr] = o[:, NMT + m]

    with np.errstate(divide="ignore", invalid="ignore"):
        q = t_p + 1.0 / r1
    d_ap = 1.0 - t_p
    d_semi = 1.0 - q
    lo = t_p - MARGIN

    # validity from class counts
    _, inv, counts = np.unique(slab, return_inverse=True, return_counts=True)
    cnt_row = counts[inv]
    valid = (cnt_row >= 2) & (cnt_row <= B - 1)

    EDGE = 1e-3
    semi_ok = (q > lo + EDGE) & (q < t_p) & np.isfinite(q) & (r1 < 0)
    redo = valid & ~semi_ok

    per_row = np.where(valid, np.maximum(d_ap - d_semi + MARGIN, 0.0), 0.0)

    if redo.any():
        e = emb / np.maximum(
            np.linalg.norm(emb, axis=1, keepdims=True), 1e-12
        )
        idx = order[np.flatnonzero(redo)]  # original row indices
        for g, i in zip(np.flatnonzero(redo), idx):
            dot = (e[i] @ e.T).astype(np.float32)
            dist = np.clip(1.0 - dot, 0.0, None)
            pos = (lab == lab[i])
            pos[i] = False
            neg = lab != lab[i]
            dap = dist[pos].max()
            semi = neg & (dist > dap) & (dist < dap + MARGIN)
            if semi.any():
                dan = dist[semi].min()
            else:
                dan = dist[neg].min()
            per_row[g] = max(dap - dan + MARGIN, 0.0)

    num_valid = max(int(valid.sum()), 1)
    loss = per_row[valid].sum() / num_valid
    return np.array(loss, dtype=np.float32)


def kernel_run(embeddings, labels, trace=False):
    import concourse.bass_utils as bass_utils

    emb, lab, plan, cores = _prepare(embeddings, labels)
    diag = plan["diag"]
    wins = plan["wins"]
    cp = plan["cp"]
    key = (
        tuple(tuple(d) for d in diag),
        tuple(tuple(w) for w in wins),
        cp,
    )
    if key not in _CACHE:
        _CACHE[key] = _build_bass(diag, wins, cp)
    nc = _CACHE[key]
    in_maps = [
        {"xt": np.ascontiguousarray(c[0]), "xb": np.ascontiguousarray(c[1]),
         "pl": np.ascontiguousarray(c[2]), "pr": np.ascontiguousarray(c[3])}
        for c in cores
    ]
    res = bass_utils.run_bass_kernel_spmd(
        nc, in_maps, core_ids=list(range(NCORES)), trace=trace
    )
    loss = _host_reduce(emb, lab, plan, res.results)
    return loss, res


def kernel(embeddings, labels):
    loss, _ = kernel_run(embeddings, labels)
    return loss


# revision 73
# speedup vs baseline: 1.4471x; 1.0172x over previous
"""Batch semi-hard triplet loss (cosine distance) on 8 Trainium2 NeuronCores.

Strategy (data-parallel over rows, per sharding hint):
  - Host: sort rows by label; normalize embeddings exactly in f32 (glue-scale
    O(B*D) work, 0.01% of the O(B^2*D) device FLOPs) and ship bf16; core c
    takes sorted rows [1024c, 1024(c+1)) in 8 exact 128-row M-tiles; columns
    rotated per core so its rows' class columns sit in the first banks.
  - Device (per core, uniform SPMD program):
      * per M-tile m: 16 bank matmuls. Class-column poison (-2) is applied
        ON THE PE via small rank-per-class accumulate matmuls (lhsT =
        -2*row-indicators, rhs = col-indicators) -- no mask adds anywhere.
      * t_p(m) (min positive-class dot) is precomputed one M-tile AHEAD
        from tiny narrow-window matmuls (+pois) and DVE window mins, so
        consumers fire the moment each bank's matmul lands; the f16
        -t_p row for the PE subtract is transposed [128,1]->[1,128] via a
        DRAM round-trip DMA.
      * threshold-max q = max{dot < t_p} via u = 1/(dot - t_p) (pole at
        the threshold flips the order; min u = dot closest below t_p):
          - Act banks 0-11: Reciprocal with per-partition -t_p bias,
            PSUM->bf16, in four 3-bank-wide streams (Act is the pacer);
          - DVE banks 12-15: PE pre-subtracts t_p (k=1 f16-row accumulate
            matmul), then a plain DVE Reciprocal (interleaved with the Act
            groups so both families pipeline independently);
          - r1 = min(u) in ONE fused DVE tensor_scalar with accum_out
            (4x mode), one M-tile behind.
        (GPSIMD can't read PSUM or do min on TRN2, so Pool sits out.)
      * per-family PSUM pools (Act 2x3 banks, DVE/mini 2x1) keep the two
        pipelines from ringing on PSUM reuse; a dummy recip preloads the
        Act table and dummy matmuls pre-warm the PE p-state during loads.
  - Host: q = t_p + 1/r1; per-row loss epilogue in f64; rows with no
    semi-hard candidate in the margin window (or near the branch boundary)
    are recomputed exactly in f32 numpy; mean over valid rows.
"""

import numpy as np
import ml_dtypes

B = 8192
D = 128
MARGIN = 0.2
NCORES = 8
NT = 512            # N-tile width (one PSUM bank of fp32)
N_NT = B // NT      # 16
MT = 128            # M-tile rows
NMT = B // NCORES // MT  # 8 m-tiles per core
GWA = 3             # banks per Act-family PSUM tile
POIS = -2.0         # class-column poison (exactly representable in bf16)

# bank families: Act does recip on banks 0..11; DVE on banks 12..15
# (GPSIMD cannot read PSUM on TRN2, so Pool is out of the main loop)
ACT_BANKS = list(range(0, 12))
DVE_BANKS = [12, 13, 14, 15]
NA = len(ACT_BANKS)
ND = len(DVE_BANKS)

BF16 = ml_dtypes.bfloat16

_CACHE = {}


# --------------------------------------------------------------------------
# host-side planning (pure layout, computed from labels)
# --------------------------------------------------------------------------
def _plan(labels: np.ndarray):
    order = np.argsort(labels, kind="stable")
    slab = labels[order]
    bounds = np.flatnonzero(np.r_[True, slab[1:] != slab[:-1], True])
    cls_start, cls_end = bounds[:-1], bounds[1:]
    row_s = np.empty(B, dtype=np.int64)
    row_e = np.empty(B, dtype=np.int64)
    for s, e in zip(cls_start, cls_end):
        row_s[s:e] = s
        row_e[s:e] = e

    rows_per_core = B // NCORES
    cores = []
    for c in range(NCORES):
        r0 = c * rows_per_core
        base = int(row_s[r0])  # start of first class -> no wraparound
        diag = []
        for m in range(NMT):
            rr = slice(r0 + m * MT, r0 + (m + 1) * MT)
            s = row_s[rr] - base
            e = row_e[rr] - base
            dts = sorted(set((s // NT).tolist()) | set(((e - 1) // NT).tolist()))
            diag.append(dts)
        cores.append(dict(r0=r0, base=base, diag=diag))
    # unify diag sets across cores so all 8 run one compiled program
    uni = [
        sorted(set().union(*[set(pc["diag"][m]) for pc in cores]))
        for m in range(NMT)
    ]
    for pc in cores:
        pc["diag"] = uni
    # per (m, diag tile): narrow column window [c0, c1) within the bank that
    # contains every class column of the tile's rows, across all cores
    wins = []
    for m in range(NMT):
        wm_ = []
        for d in uni[m]:
            c0, c1 = NT, 0
            for pc in cores:
                rr = slice(pc["r0"] + m * MT, pc["r0"] + (m + 1) * MT)
                s = np.maximum(row_s[rr] - pc["base"] - d * NT, 0)
                e = np.minimum(row_e[rr] - pc["base"] - d * NT, NT)
                ok = s < e
                if ok.any():
                    c0 = min(c0, int(s[ok].min()))
                    c1 = max(c1, int(e[ok].max()))
            if c1 <= c0:
                c0, c1 = 0, NT
            wm_.append((c0, c1))
        wins.append(wm_)
    # max classes per (m, diag-tile) block across cores (pois matmul k-dim)
    cp = 1
    for c in range(NCORES):
        pc = cores[c]
        r0, base = pc["r0"], pc["base"]
        for m in range(NMT):
            rr = slice(r0 + m * MT, r0 + (m + 1) * MT)
            ss = row_s[rr]
            for d in uni[m]:
                lo, hi = base + d * NT, base + (d + 1) * NT
                # classes whose column range intersects the bank
                cls = set()
                for g in range(rr.start, rr.stop):
                    if row_s[g] < hi and row_e[g] > lo:
                        cls.add(int(row_s[g]))
                cp = max(cp, len(cls))
    return dict(
        order=order, row_s=row_s, row_e=row_e, cores=cores, diag=uni,
        wins=wins, cp=cp,
    )


def _build_core_inputs(emb_norm: np.ndarray, plan, c: int):
    """emb_norm: label-sorted, unit-normalized embeddings (f32).
    Returns (xtn_rot [D,B], xbn [D,1024], pl [CP, nblk*MT],
    pr [CP, nblk*NT]) all bf16."""
    pc = plan["cores"][c]
    base, r0 = pc["base"], pc["r0"]
    rows_per_core = B // NCORES
    row_s, row_e = plan["row_s"], plan["row_e"]
    cp = plan["cp"]

    rot = np.r_[np.arange(base, B), np.arange(0, base)]
    xt_rot = np.ascontiguousarray(emb_norm[rot].T).astype(BF16)
    xb = np.ascontiguousarray(emb_norm[r0 : r0 + rows_per_core].T).astype(BF16)

    # poison matmul blocks: per (m, d in diag[m]):
    #   pl[k, i] = -2 if m-tile row i in class k else 0     [CP, MT]
    #   pr[k, j] = 1 if bank-d col j in class k else 0      [CP, NT]
    nblk = sum(len(d) for d in pc["diag"])
    pl = np.zeros((cp, nblk * MT), np.float32)
    pr = np.zeros((cp, nblk * NT), np.float32)
    bi = 0
    for m in range(NMT):
        for d in pc["diag"][m]:
            lo, hi = base + d * NT, base + (d + 1) * NT
            cls = {}
            for r in range(MT):
                g = r0 + m * MT + r
                s, e = int(row_s[g]), int(row_e[g])
                if s < hi and e > lo:
                    k = cls.setdefault(s, len(cls))
                    pl[k, bi * MT + r] = POIS
                    cs, ce = max(s - lo, 0), min(e - lo, NT)
                    pr[k, bi * NT + cs : bi * NT + ce] = 1.0
            assert len(cls) <= cp
            bi += 1
    pl = pl.astype(BF16)
    pr = pr.astype(BF16)

    return xt_rot, xb, pl, pr


# --------------------------------------------------------------------------
# device program
# --------------------------------------------------------------------------
def _raw_recip_bias(nc, out, in_, bias_ap):
    import concourse.mybir as mybir

    eng = nc.scalar
    ins = [
        eng.lower_ap(in_),
        eng.lower_ap(bias_ap),
        mybir.ImmediateValue(dtype=mybir.dt.float32, value=1.0),  # scale
        mybir.ImmediateValue(dtype=mybir.dt.float32, value=0.0),  # alpha
    ]
    return eng.add_instruction(
        mybir.InstActivation(
            name=f"I-{nc.next_id()}",
            func=mybir.ActivationFunctionType.Reciprocal,
            ins=ins,
            outs=[eng.lower_ap(out)],
        )
    )


def _build_bass(diag, wins, cp):
    import concourse.bacc as bacc
    import concourse.mybir as mybir
    from concourse.tile import TileContext

    f32 = mybir.dt.float32
    bf16 = mybir.dt.bfloat16
    f16 = mybir.dt.float16
    Alu = mybir.AluOpType
    NBC = NMT * MT  # xb columns (1024)
    nblk = sum(len(d) for d in diag)

    nc = bacc.Bacc("TRN2", target_bir_lowering=False, debug=False, num_devices=NCORES)

    xt_d = nc.dram_tensor("xt", [D, B], bf16, kind="ExternalInput").ap()
    xb_d = nc.dram_tensor("xb", [D, NBC], bf16, kind="ExternalInput").ap()
    pl_d = nc.dram_tensor("pl", [cp, nblk * MT], bf16, kind="ExternalInput").ap()
    pr_d = nc.dram_tensor("pr", [cp, nblk * NT], bf16, kind="ExternalInput").ap()
    out_d = nc.dram_tensor("out", [MT, 2 * NMT], f32, kind="ExternalOutput").ap()
    # per-m transposed -t_p rows (f16) round-trip through DRAM
    scr_d = nc.dram_tensor("scr", [NMT, MT], f16, kind="Internal").ap()

    # diag-block flat index per (m, d)
    blkof = {}
    bi = 0
    for m in range(NMT):
        for j, d in enumerate(diag[m]):
            blkof[(m, d)] = bi
            bi += 1

    with TileContext(nc) as tc:
        with (
            tc.tile_pool(name="big", bufs=1) as big,
            tc.tile_pool(name="upool", bufs=2) as upool,
            tc.tile_pool(name="sm", bufs=6) as smp,
            tc.tile_pool(name="psa", bufs=2, space="PSUM") as psa,
            tc.tile_pool(name="psp", bufs=2, space="PSUM") as psw,
        ):
            # ---------------- setup: load pre-normalized inputs -------------
            # dummy recip FIRST so its act-table load isn't queued behind
            # the Act-queue DMA issues below
            dum = big.tile([1, 1], f32, tag="dum")
            nc.vector.memset(dum[:], 1.0)
            dumo = big.tile([1, 1], bf16, tag="dumo")
            _raw_recip_bias(nc, dumo[:], dum[:], dum[:])
            # xbn + first xtn chunks on the SP HWDGE queue; the back half of
            # xtn on the Activation HWDGE queue (two queues load in parallel)
            xbn = big.tile([D, NBC], bf16, tag="xbn")
            nc.sync.dma_start(xbn[:], xb_d)
            pl = big.tile([cp, nblk * MT], bf16, tag="pl")
            nc.sync.dma_start(pl[:], pl_d)
            pr = big.tile([cp, nblk * NT], bf16, tag="pr")
            nc.sync.dma_start(pr[:], pr_d)
            xtn = big.tile([D, B], bf16, tag="xtn")
            for j in range(4):
                sl = slice(j * (B // 8), (j + 1) * (B // 8))
                nc.sync.dma_start(xtn[:, sl], xt_d[:, sl])
            for j in range(4, 8):
                sl = slice(j * (B // 8), (j + 1) * (B // 8))
                nc.scalar.dma_start(xtn[:, sl], xt_d[:, sl])

            outb = big.tile([MT, 2 * NMT], f32, tag="outb")
            ntpall = big.tile([MT, NMT], f32, tag="ntpall")
            ones = big.tile([1, NT], f16, tag="ones")
            nc.vector.memset(ones[:], 1.0)
            onesr = big.tile([1, MT], f16, tag="onesr")
            nc.vector.memset(onesr[:], 1.0)
            # pre-warm the PE p-state with a ~3us dummy matmul streak
            for _ in range(2):
                warm = psw.tile([MT, NT], f32, tag="w")
                for _k in range(7):
                    nc.tensor.matmul(warm[:], onesr[:], ones[:])

            # ---------------- main loop over M-tiles ----------------
            # Act-family groups of GWA banks; DVE-family single banks
            agroups = [
                ACT_BANKS[i : i + GWA] for i in range(0, NA, GWA)
            ]

            def emit_tp(m):
                """Narrow-window diag matmuls (+pois) into small PSUM tiles,
                then the t_p chain on DVE."""
                dts = diag[m]
                lhsT = xbn[:, m * MT : (m + 1) * MT]
                tpp = outb[:, m : m + 1]
                ntp = ntpall[:, m : m + 1]
                minis = []
                for j, d in enumerate(dts):
                    mini = psw.tile([MT, NT], f32, tag="w")
                    minis.append(mini)
                    c0, c1 = wins[m][j]
                    w = c1 - c0
                    i = blkof[(m, d)]
                    nc.tensor.matmul(
                        mini[:, 0:w],
                        lhsT, xtn[:, d * NT + c0 : d * NT + c1],
                        start=True, stop=False,
                    )
                    nc.tensor.matmul(
                        mini[:, 0:w],
                        pl[:, i * MT : (i + 1) * MT],
                        pr[:, i * NT + c0 : i * NT + c1],
                        start=False, stop=True,
                    )
                ndts = len(dts)
                posm = smp.tile([MT, max(ndts, 1)], f32, tag="posm")
                for j, d in enumerate(dts):
                    c0, c1 = wins[m][j]
                    nc.vector.tensor_reduce(
                        posm[:, j : j + 1],
                        minis[j][:, 0 : c1 - c0],
                        axis=mybir.AxisListType.X, op=Alu.min,
                    )
                if ndts == 1:
                    minpos = posm[:, 0:1]
                else:
                    mp = smp.tile([MT, 1], f32, tag="minpos")
                    nc.vector.tensor_reduce(
                        mp[:], posm[:], axis=mybir.AxisListType.X, op=Alu.min
                    )
                    minpos = mp[:]
                # t_p = min(minpos - POIS, 1)
                nc.vector.tensor_scalar(
                    tpp, minpos, -POIS, 1.0, Alu.add, Alu.min
                )
                nc.vector.tensor_scalar_mul(ntp, tpp, -1.0)
                # transposed -t_p row (f16) for the PE subtract on DVE banks:
                # col -> row via a DRAM round-trip (2 tiny DMAs, 1 m ahead)
                ntph = smp.tile([MT, 1], f16, tag="ntph", bufs=3)
                with nc.allow_low_precision(reason="f16 threshold row"):
                    nc.vector.tensor_scalar_mul(ntph[:], tpp, -1.0)
                nc.sync.dma_start(scr_d[m : m + 1, :], ntph[:])
                tprow = smp.tile([1, MT], f16, tag="tprow", bufs=3)
                nc.sync.dma_start(tprow[:], scr_d[m : m + 1, :])
                tprows[m] = tprow

            def mm_bank(wg, ofs, t, m, lhsT, stop=True):
                """main matmul for N-tile t into wg[:, ofs*NT:...], plus the
                class-poison accumulate matmul on diag banks."""
                if t in diag[m]:
                    nc.tensor.matmul(
                        wg[:, ofs * NT : (ofs + 1) * NT],
                        lhsT, xtn[:, t * NT : (t + 1) * NT],
                        start=True, stop=False,
                    )
                    i = blkof[(m, t)]
                    nc.tensor.matmul(
                        wg[:, ofs * NT : (ofs + 1) * NT],
                        pl[:, i * MT : (i + 1) * MT],
                        pr[:, i * NT : (i + 1) * NT],
                        start=False, stop=stop,
                    )
                else:
                    nc.tensor.matmul(
                        wg[:, ofs * NT : (ofs + 1) * NT],
                        lhsT, xtn[:, t * NT : (t + 1) * NT],
                        start=True, stop=stop,
                    )

            tprows = {}
            emit_tp(0)

            pending_min = None  # (u, m) of previous M-tile

            def emit_mins(u, m):
                # fused elementwise+accumulate-min: r1 = min(u) on DVE (4x)
                uj = upool.tile([MT, N_NT * NT], bf16, tag="uj")
                with nc.allow_low_precision(reason="u is bf16 by design"):
                    nc.vector.tensor_scalar(
                        uj[:], u[:], 1.0, None, Alu.mult, Alu.min,
                        accum_out=outb[:, NMT + m : NMT + m + 1],
                    )

            for m in range(NMT):
                lhsT = xbn[:, m * MT : (m + 1) * MT]
                ntp = ntpall[:, m : m + 1]     # -t_p ptr (f32)
                u = upool.tile([MT, N_NT * NT], bf16, tag="u")

                def emit_a(banks):
                    wg = psa.tile([MT, GWA * NT], f32, tag="w")
                    for k, t in enumerate(banks):
                        mm_bank(wg, k, t, m, lhsT)
                    b0 = banks[0]
                    _raw_recip_bias(
                        nc,
                        u[:, b0 * NT : (b0 + len(banks)) * NT],
                        wg[:, 0 : len(banks) * NT],
                        ntp,
                    )

                def emit_d(t):
                    # PE pre-subtracts t_p (k=1 f16 row), then DVE reciprocal
                    wg = psw.tile([MT, NT], f32, tag="w")
                    mm_bank(wg, 0, t, m, lhsT, stop=False)
                    nc.tensor.matmul(
                        wg[:], tprows[m][:], ones[:],
                        start=False, stop=True,
                    )
                    with nc.allow_low_precision(reason="u is bf16 by design"):
                        nc.vector.reciprocal(
                            u[:, t * NT : (t + 1) * NT], wg[:]
                        )

                # interleave the two families so Act and DVE pipeline
                # independently and the PE weaves between them; the next
                # M-tile's t_p chain is issued early so its minis aren't
                # stuck behind all of this tile's fills on the PE queue
                emit_a(agroups[0])
                if m + 1 < NMT:
                    emit_tp(m + 1)
                emit_d(DVE_BANKS[0])
                emit_a(agroups[1])
                emit_d(DVE_BANKS[1])
                emit_a(agroups[2])
                emit_d(DVE_BANKS[2])
                emit_a(agroups[3])
                emit_d(DVE_BANKS[3])

                if pending_min is not None:
                    emit_mins(*pending_min)
                pending_min = (u, m)

            # final M-tile: split the min so the first half overlaps the
            # tail recips, shrinking the serial epilogue
            u, m = pending_min
            H = 9 * NT  # banks 0..8 done after the first three Act groups
            uj = upool.tile([MT, N_NT * NT], bf16, tag="uj")
            ra = smp.tile([MT, 2], f32, tag="ra")
            with nc.allow_low_precision(reason="u is bf16 by design"):
                nc.vector.tensor_scalar(
                    uj[:, 0:H], u[:, 0:H], 1.0, None, Alu.mult, Alu.min,
                    accum_out=ra[:, 0:1],
                )
                nc.vector.tensor_scalar(
                    uj[:, H:], u[:, H:], 1.0, None, Alu.mult, Alu.min,
                    accum_out=ra[:, 1:2],
                )
            nc.vector.tensor_tensor(
                outb[:, NMT + m : NMT + m + 1], ra[:, 0:1], ra[:, 1:2],
                Alu.min,
            )

            nc.sync.dma_start(out_d, outb[:])

    nc.compile()
    return nc


# --------------------------------------------------------------------------
# entry point
# --------------------------------------------------------------------------
def _prepare(embeddings, labels):
    emb = np.asarray(embeddings, dtype=np.float32)
    lab = np.asarray(labels).astype(np.int64)
    plan = _plan(lab)
    emb_sorted = emb[plan["order"]]
    norm = np.linalg.norm(emb_sorted, axis=1, keepdims=True)
    emb_norm = emb_sorted / np.maximum(norm, 1e-12)
    cores = [_build_core_inputs(emb_norm, plan, c) for c in range(NCORES)]
    return emb, lab, plan, cores


def _host_reduce(emb, lab, plan, outs):
    """outs: per core {"out": [128, 16] f32} (cols 0-7 t_p, 8-15 r1)."""
    order = plan["order"]
    slab = lab[order]
    rows_per_core = B // NCORES

    t_p = np.zeros(B, np.float64)
    r1 = np.zeros(B, np.float64)
    for c in range(NCORES):
        o = np.asarray(outs[c]["out"], np.float64)
        for m in range(NMT):
            rr = slice(c * rows_per_core + m * MT, c * rows_per_core + (m + 1) * MT)
            t_p[rr] = o[:, m]
            r1[rr] = o[:, NMT + m]

    with np.errstate(divide="ignore", invalid="ignore"):
        q = t_p + 1.0 / r1
    d_ap = 1.0 - t_p
    d_semi = 1.0 - q
    lo = t_p - MARGIN

    # validity from class counts
    _, inv, counts = np.unique(slab, return_inverse=True, return_counts=True)
    cnt_row = counts[inv]
    valid = (cnt_row >= 2) & (cnt_row <= B - 1)

    EDGE = 1e-3
    semi_ok = (q > lo + EDGE) & (q < t_p) & np.isfinite(q) & (r1 < 0)
    redo = valid & ~semi_ok

    per_row = np.where(valid, np.maximum(d_ap - d_semi + MARGIN, 0.0), 0.0)

    if redo.any():
        e = emb / np.maximum(
            np.linalg.norm(emb, axis=1, keepdims=True), 1e-12
        )
        idx = order[np.flatnonzero(redo)]  # original row indices
        for g, i in zip(np.flatnonzero(redo), idx):
            dot = (e[i] @ e.T).astype(np.float32)
            dist = np.clip(1.0 - dot, 0.0, None)
            pos = (lab == lab[i])
            pos[i] = False
            neg = lab != lab[i]
            dap = dist[pos].max()
            semi = neg & (dist > dap) & (dist < dap + MARGIN)
            if semi.any():
                dan = dist[semi].min()
            else:
                dan = dist[neg].min()
            per_row[g] = max(dap - dan + MARGIN, 0.0)

    num_valid = max(int(valid.sum()), 1)
    loss = per_row[valid].sum() / num_valid
    return np.array(loss, dtype=np.float32)


def kernel_run(embeddings, labels, trace=False):
    import concourse.bass_utils as bass_utils

    emb, lab, plan, cores = _prepare(embeddings, labels)
    diag = plan["diag"]
    wins = plan["wins"]
    cp = plan["cp"]
    key = (
        tuple(tuple(d) for d in diag),
        tuple(tuple(w) for w in wins),
        cp,
    )
    if key not in _CACHE:
        _CACHE[key] = _build_bass(diag, wins, cp)
    nc = _CACHE[key]
    in_maps = [
        {"xt": np.ascontiguousarray(c[0]), "xb": np.ascontiguousarray(c[1]),
         "pl": np.ascontiguousarray(c[2]), "pr": np.ascontiguousarray(c[3])}
        for c in cores
    ]
    res = bass_utils.run_bass_kernel_spmd(
        nc, in_maps, core_ids=list(range(NCORES)), trace=trace
    )
    loss = _host_reduce(emb, lab, plan, res.results)
    return loss, res


def kernel(embeddings, labels):
    loss, _ = kernel_run(embeddings, labels)
    return loss
